# revision 32
# baseline (speedup 1.0000x reference)
"""Single-head causal attention on 8 Trainium2 NeuronCores.

Problem: x:[8,2048,1024], Wq/Wk/Wv:[64,1024], bq/bk/bv:[64]
  q,k,v = x@W*.T + b*;  out = softmax(causal(q@k.T)/sqrt(64)) @ v

Sharding: batch dim (8) across the 8 cores - fully data-parallel, no
collectives. Each core computes one batch's attention head.

Per-core design (all matmuls fp16; fp16 has the same 10-bit mantissa as
tf32, so scores keep tf32-grade accuracy at half the DMA bytes):
  - host supplies xT [E,S] fp16; [Wq/8 | Wk] packed fp16 [E,128] (q scale
    folded into Wq/bq), WvT fp16 [E,64].
  - q-chunk widths are parameterized (WIDTHS); uniform 512 measured best.
  - qk projection: ONE accumulating matmul group per chunk into psum
    [128,W] (q rows 0:64, k rows 64:128); epilogue = single DVE
    tensor_scalar (bias add, fp16 out). k is re-based to partitions 0:64
    with a cheap SBUF->SBUF DMA; only the chunk's own (band) score tiles
    wait on it - earlier k-tiles come from previous chunks.
  - v projection FLIPPED: out [128 s, 64 h] with the x tile as the
    stationary operand (cost = 64 rows/matmul, 4x cheaper than h-major),
    landing directly in the k-major layout the AV matmul needs - no PE
    transposes, no re-pack copies. v bias is NOT applied on device: with
    denominator d = sum(w), sum(w (v+bv))/d = sum(w v)/d + bv, so the host
    adds bv after normalizing.
  - scores per k-tile PAIR into a [128,2W] psum tile; ONE exp instruction
    covers both halves (saves the per-instruction ACT fixed cost). For
    512-wide chunks the second diagonal-band pair computes/exps only
    columns 256:512 (strided [128,2,256] subrect).
  - causal masking: only the 128-col partial strip of each diagonal tile
    needs a mask; all strips share ONE [128,128] ramp (j>=p), applied as a
    fp16 DVE multiply (2x mode). Dead regions are never read because AV
    accumulates per q-subtile.
  - AV FLIPPED: out [128 q, 65 h] per 128-row q-subtile, accumulating
    lhsT = w[:,128-col slice] x rhs = vsb[128 k, 65] over k-tiles. Cost is
    65 rows/matmul. The ones column in vsb makes out[:,64] the softmax
    denominator (divided on host).
  - out [S, 65] fp32, DMA'd per chunk.
  - DMA queues: sync = startup x + consts + rebases (kept short so the
    greedy scheduler can't bury critical transfers behind prefetch);
    gpsimd = weights + bulk x prefetch (SWDGE starts transfers ~100ns in)
    + out; scalar = wv + one startup x tile + final out half.
"""

import numpy as np

import concourse.bacc as bacc
import concourse.mybir as mybir
import concourse.tile as tile
from concourse import bass2jax

B, S, E, H = 8, 2048, 1024, 64
NCORES = 8
PB = 128  # partition block / k-tile / q-subtile size
ET = E // PB  # e-tiles per contraction
KT = S // PB  # k-tiles / q-subtiles

# q-chunk widths (must sum to S, each a multiple of PB, max 512)
WIDTHS = [512, 512, 512, 512]
CHUNKS = []
_c0 = 0
for _w in WIDTHS:
    CHUNKS.append((_c0, _w))
    _c0 += _w
assert _c0 == S
QC = len(CHUNKS)

# consts layout (f32 columns; fp16 payloads are bit-packed in pairs so the
# tensor stays NaN-free for finite checks)
C_QKB = 0  # [*, 0:1]   qk bias f32 (rows 0:64 bq/8, 64:128 bk)
C_MASK = 1  # [*, 1:65]  causal ramp strip, fp16 pairs: (p, j) = 1 iff j >= p
C_ONES = C_MASK + PB // 2  # [*, 65:66] fp16 ones pair
NCONST = C_ONES + 1

F32 = mybir.dt.float32
F16 = mybir.dt.float16
AF = mybir.ActivationFunctionType
MUL = mybir.AluOpType.mult

_CACHE: dict = {}

CFG = {
    "xbufs": 12,
    "wbufs": 10,
    "dma2": "gpsimd",
    # hold chunk-c x prefetch DMAs until ~these sim times (ms): keeps the
    # greedy scheduler from stuffing in-order DMA queues with prefetch
    # ahead of critical-path transfers
    "xwait": [0.0, 0.0, 0.002, 0.005, 0.009],
}


def _interleave(*gens):
    """Drive generators round-robin, one step per turn, in priority order."""
    alive = list(gens)
    while alive:
        for g in list(alive):
            try:
                next(g)
            except StopIteration:
                alive.remove(g)


def _build_nc():
    nc = bacc.Bacc("TRN2", target_bir_lowering=False, debug=False)
    xT = nc.dram_tensor("xT", [E, S], F16, kind="ExternalInput").ap()
    wqk = nc.dram_tensor("wqk", [E, PB], F16, kind="ExternalInput").ap()
    wv = nc.dram_tensor("wv", [E, H], F16, kind="ExternalInput").ap()
    consts = nc.dram_tensor("consts", [PB, NCONST], F32, kind="ExternalInput").ap()
    out = nc.dram_tensor("out", [S, H + 1], F32, kind="ExternalOutput").ap()

    with tile.TileContext(nc) as tc:
        with (
            tc.tile_pool(name="const", bufs=1) as constp,
            tc.tile_pool(name="xs", bufs=CFG["xbufs"]) as xpool,
            tc.tile_pool(name="qkv", bufs=1) as qkvp,
            tc.tile_pool(name="wt", bufs=CFG["wbufs"]) as wtp,
            tc.tile_pool(name="pqk", bufs=1, space="PSUM") as pqk,
            tc.tile_pool(name="pv", bufs=1, space="PSUM") as pvp,
            tc.tile_pool(name="ps", bufs=2, space="PSUM") as psp,
            tc.tile_pool(name="pavA", bufs=1, space="PSUM") as pavA,
            tc.tile_pool(name="pavB", bufs=1, space="PSUM") as pavB,
        ):
            wqk_sb = constp.tile([PB, ET, PB], F16)
            wv_sb = constp.tile([PB, ET, H], F16)
            cs = constp.tile([PB, NCONST], F32)
            # first weight slice gates the first matmul; gpsimd/SWDGE
            # starts transfers ~100ns into the kernel
            nc.gpsimd.dma_start(
                wqk_sb[:, 0:2, :],
                wqk[0 : 2 * PB, :].rearrange("(t p) m -> p t m", p=PB),
            )

            def load_weights_rest():
                nc.gpsimd.dma_start(
                    wqk_sb[:, 2:4, :],
                    wqk[2 * PB : 4 * PB, :].rearrange("(t p) m -> p t m", p=PB),
                )
                nc.gpsimd.dma_start(
                    wqk_sb[:, 4:ET, :],
                    wqk[4 * PB :, :].rearrange("(t p) m -> p t m", p=PB),
                )
                nc.scalar.dma_start(
                    wv_sb[:], wv[:].rearrange("(t p) m -> p t m", p=PB)
                )

            qkbias_ap = cs[:, C_QKB : C_QKB + 1]
            mask_ap = cs[:, C_MASK : C_MASK + PB // 2].bitcast(F16)

            qk_sb = qkvp.tile([PB, S], F16)  # q at 0:64, k at 64:128
            kT = qkvp.tile([H, S], F16)  # k re-based to partitions 0:64
            vsb = qkvp.tile([PB, KT, H + 1], F16)  # v k-major + ones col
            osb = qkvp.tile([PB, KT, H + 1], F32)  # out staging

            def load_consts():
                nc.sync.dma_start(cs[:], consts[:])
                ones16 = cs[:, C_ONES : C_ONES + 1].bitcast(F16)
                nc.vector.tensor_copy(
                    vsb[:, :, H : H + 1],
                    ones16[:, 0:1, None].to_broadcast((PB, KT, 1)),
                )

            proj_state = {}

            def proj_main(ci):
                col0, W = CHUNKS[ci]
                qs = slice(col0, col0 + W)
                p_qk = pqk.tile([PB, W], F32, tag="pqk")
                xts = []
                with tc.tile_wait_until(CFG["xwait"][ci]):
                    for ep in range(ET // 2):
                        xt = xpool.tile([PB, 2, W], F16, tag="xt")
                        xts.append(xt)
                        if ci == 0:
                            dma_eng = (nc.sync, getattr(nc, CFG["dma2"]), nc.scalar, nc.sync)[ep]
                        elif ci == 1:
                            dma_eng = nc.sync if ep % 2 == 0 else getattr(nc, CFG["dma2"])
                        else:
                            dma_eng = getattr(nc, CFG["dma2"])
                        dma_eng.dma_start(
                            xt[:],
                            xT[ep * 2 * PB : (ep + 1) * 2 * PB, qs].rearrange(
                                "(t p) q -> p t q", p=PB
                            ),
                        )
                for ep in range(ET // 2):
                    for t in range(2):
                        e = 2 * ep + t
                        nc.tensor.matmul(
                            p_qk[:],
                            wqk_sb[:, e, :],
                            xts[ep][:, t, :],
                            start=(e == 0),
                            stop=(e == ET - 1),
                        )
                    yield
                proj_state[ci] = (p_qk, xts)

            def epi_qk(ci):
                col0, W = CHUNKS[ci]
                qs = slice(col0, col0 + W)
                p_qk, xts = proj_state[ci]
                if ci == 0:
                    # ACT is idle at startup (DVE does the ones-fill)
                    nc.scalar.activation(
                        qk_sb[:, qs], p_qk[:], AF.Identity, bias=qkbias_ap
                    )
                else:
                    nc.vector.tensor_scalar(
                        qk_sb[:, qs], p_qk[:], qkbias_ap, None,
                        mybir.AluOpType.add, mybir.AluOpType.bypass,
                    )
                # re-base k to partitions 0:64 for the scores matmul; only
                # this chunk's own (band) score tiles wait on it
                nc.sync.dma_start(kT[:, qs], qk_sb[H:PB, qs])

            def v_flip(ci):
                # flipped v projection: out [128 s, 64 h], x slice stationary
                col0, W = CHUNKS[ci]
                _, xts = proj_state[ci]
                for sub in range(W // PB):
                    m = col0 // PB + sub
                    p_v = pvp.tile([PB, H], F32, tag="pv")
                    for e in range(ET):
                        nc.tensor.matmul(
                            p_v[:],
                            xts[e // 2][:, e % 2, sub * PB : (sub + 1) * PB],
                            wv_sb[:, e, :],
                            start=(e == 0),
                            stop=(e == ET - 1),
                        )
                    nc.vector.tensor_copy(vsb[:, m, 0:H], p_v[:])
                    yield
                proj_state.pop(ci)

            def attn(ci):
                col0, W = CHUNKS[ci]
                nkt = (col0 + W) // PB  # k-tiles this chunk attends to
                nband = W // PB  # its own (diagonal band) k-tiles
                npair = nkt // 2
                kt0 = col0 // PB  # first band k-tile / q-subtile
                ws = {}

                def score_exp_pair(p, lo=0):
                    # pair p covers k-tiles (2p, 2p+1); lo>0 computes/exps
                    # only columns lo:W (band subrect)
                    p_s = psp.tile([PB, 2 * W], F32, tag="ps")
                    w = wtp.tile([PB, 2 * W], F16, tag="w")
                    for t in range(2):
                        m = 2 * p + t
                        nc.tensor.matmul(
                            p_s[:, t * W + lo : (t + 1) * W],
                            kT[:, m * PB : (m + 1) * PB],
                            qk_sb[0:H, col0 + lo : col0 + W],
                            start=True,
                            stop=True,
                        )
                    if lo:
                        nc.scalar.activation(
                            w[:].rearrange("p (t q) -> p t q", t=2)[:, :, lo:W],
                            p_s[:].rearrange("p (t q) -> p t q", t=2)[:, :, lo:W],
                            AF.Exp,
                        )
                    else:
                        nc.scalar.activation(w[:], p_s[:], AF.Exp)
                    ws[p] = w

                def strip(d):
                    # diagonal strip mask: first 128 valid cols of band tile d
                    m = kt0 + d
                    colo = (m % 2) * W + d * PB
                    w = ws[m // 2]
                    nc.vector.tensor_tensor(
                        w[:, colo : colo + PB],
                        w[:, colo : colo + PB],
                        mask_ap,
                        MUL,
                    )

                av_state = {}

                def av(sub, until=None):
                    # flipped AV: per q-subtile accumulation, out [128 q, 65].
                    # until=kt emits only tiles < kt (group left open); a
                    # later av(sub) call finishes and copies out.
                    s = kt0 + sub
                    if sub in av_state:
                        p_av, k_from = av_state.pop(sub)
                    else:
                        p_av = (pavA if sub % 2 == 0 else pavB).tile(
                            [PB, H + 1], F32, tag="pav"
                        )
                        k_from = 0
                    hi = s + 1 if until is None else until
                    for kt in range(k_from, hi):
                        w = ws[kt // 2]
                        colo = (kt % 2) * W + sub * PB
                        nc.tensor.matmul(
                            p_av[:],
                            w[:, colo : colo + PB],
                            vsb[:, kt, :],
                            start=(kt == 0),
                            stop=(kt == s),
                        )
                    if until is not None:
                        av_state[sub] = (p_av, hi)
                        return
                    nc.vector.tensor_copy(osb[:, s, :], p_av[:])

                # non-band pairs first (they only need OLD k-tiles), then
                # band pairs; for 512-wide chunks the last band pair is a
                # subrect. Strips/AV follow their pairs.
                for p in range(npair - nband // 2):
                    score_exp_pair(p)
                    yield
                if nband == 2:
                    score_exp_pair(npair - 1)
                    yield
                    strip(0)
                    strip(1)
                    yield
                    av(0)
                    yield
                    av(1)
                    yield
                else:
                    score_exp_pair(npair - 2)
                    strip(0)
                    strip(1)
                    yield
                    av(0)
                    yield
                    av(1)
                    yield
                    score_exp_pair(npair - 1, lo=W // 2)
                    strip(2)
                    strip(3)
                    yield
                    av(2)
                    yield
                    av(3)
                    yield
                # out DMA for this chunk (final chunk split for drain overlap)
                nsplit = 2 if ci == QC - 1 else 1
                nsub = W // PB
                for hh in range(nsplit):
                    r0 = col0 + hh * (W // nsplit)
                    r1 = col0 + (hh + 1) * (W // nsplit)
                    tl = slice(kt0 + hh * (nsub // nsplit),
                               kt0 + (hh + 1) * (nsub // nsplit))
                    if ci < QC - 1:
                        oq = nc.gpsimd
                    else:
                        oq = nc.sync if hh == 0 else nc.scalar
                    oq.dma_start(
                        out[r0:r1, :].rearrange("(t p) h -> p t h", p=PB),
                        osb[:, tl, :],
                    )
                    yield

            g0 = proj_main(0)
            next(g0)  # emits chunk-0 x DMAs + first matmul pair
            load_weights_rest()
            load_consts()
            _interleave(g0)
            epi_qk(0)
            for c in range(1, QC):
                _interleave(attn(c - 1), proj_main(c), v_flip(c - 1))
                epi_qk(c)
            _interleave(attn(QC - 1), v_flip(QC - 1))

    nc.compile()
    return nc


def _host_inputs(x, Wq, bq, Wk, bk, Wv, bv):
    x = np.asarray(x, np.float32)
    Wq, bq = np.asarray(Wq, np.float32), np.asarray(bq, np.float32)
    Wk, bk = np.asarray(Wk, np.float32), np.asarray(bk, np.float32)
    Wv = np.asarray(Wv, np.float32)

    sc = np.float32(1.0 / np.sqrt(H))
    wqk_h = np.concatenate([Wq.T * sc, Wk.T], axis=1).astype(np.float16)
    wvT = np.ascontiguousarray(Wv.T).astype(np.float16)

    cs = np.zeros((PB, NCONST), np.float32)
    cs[:, C_QKB] = np.concatenate([bq * sc, bk]).astype(np.float32)
    j = np.arange(PB)[None, :]
    p = np.arange(PB)[:, None]
    mask16 = (j >= p).astype(np.float16)  # [128, 128]
    cs[:, C_MASK : C_MASK + PB // 2] = mask16.view(np.float32)
    cs[:, C_ONES] = np.array([1.0, 1.0], np.float16).view(np.float32)[0]

    shared = {"wqk": np.ascontiguousarray(wqk_h), "wv": wvT, "consts": cs}
    in_maps = []
    for b in range(B):
        m = dict(shared)
        m["xT"] = np.ascontiguousarray(x[b].T.astype(np.float16))
        in_maps.append(m)
    return in_maps


def get_nc():
    if "nc" not in _CACHE:
        _CACHE["nc"] = _build_nc()
    return _CACHE["nc"]


def kernel(x, Wq, bq, Wk, bk, Wv, bv):
    nc = get_nc()
    in_maps = _host_inputs(x, Wq, bq, Wk, bk, Wv, bv)
    results = bass2jax.run_bass_via_pjrt(nc, in_maps, n_cores=NCORES)
    bv32 = np.asarray(bv, np.float32)
    out = np.empty((B, S, H), np.float32)
    for b in range(B):
        o = results[b]["out"]
        out[b] = o[:, :H] / o[:, H : H + 1] + bv32
    return out


# revision 36
# speedup vs baseline: 1.0019x; 1.0019x over previous
"""Single-head causal attention on 8 Trainium2 NeuronCores.

Problem: x:[8,2048,1024], Wq/Wk/Wv:[64,1024], bq/bk/bv:[64]
  q,k,v = x@W*.T + b*;  out = softmax(causal(q@k.T)/sqrt(64)) @ v

Sharding: batch dim (8) across the 8 cores - fully data-parallel, no
collectives. Each core computes one batch's attention head.

Per-core design (all matmuls fp16; fp16 has the same 10-bit mantissa as
tf32, so scores keep tf32-grade accuracy at half the DMA bytes):
  - host supplies xT [E,S] fp16; [Wq/8 | Wk] packed fp16 [E,128] (q scale
    folded into Wq/bq), WvT fp16 [E,64].
  - q-chunk widths are parameterized (WIDTHS); uniform 512 measured best.
  - qk projection: ONE accumulating matmul group per chunk into psum
    [128,W] (q rows 0:64, k rows 64:128); epilogue = single DVE
    tensor_scalar (bias add, fp16 out). k is re-based to partitions 0:64
    with a cheap SBUF->SBUF DMA; only the chunk's own (band) score tiles
    wait on it - earlier k-tiles come from previous chunks.
  - v projection FLIPPED: out [128 s, 64 h] with the x tile as the
    stationary operand (cost = 64 rows/matmul, 4x cheaper than h-major),
    landing directly in the k-major layout the AV matmul needs - no PE
    transposes, no re-pack copies. v bias is NOT applied on device: with
    denominator d = sum(w), sum(w (v+bv))/d = sum(w v)/d + bv, so the host
    adds bv after normalizing.
  - scores per k-tile PAIR into a [128,2W] psum tile; ONE exp instruction
    covers both halves (saves the per-instruction ACT fixed cost). For
    512-wide chunks the second diagonal-band pair computes/exps only
    columns 256:512 (strided [128,2,256] subrect).
  - causal masking: only the 128-col partial strip of each diagonal tile
    needs a mask; all strips share ONE [128,128] ramp (j>=p), applied as a
    fp16 DVE multiply (2x mode). Dead regions are never read because AV
    accumulates per q-subtile.
  - AV FLIPPED: out [128 q, 65 h] per 128-row q-subtile, accumulating
    lhsT = w[:,128-col slice] x rhs = vsb[128 k, 65] over k-tiles. Cost is
    65 rows/matmul. The ones column in vsb makes out[:,64] the softmax
    denominator (divided on host).
  - out [S, 65] fp32, DMA'd per chunk.
  - DMA queues: sync = startup x + consts + rebases (kept short so the
    greedy scheduler can't bury critical transfers behind prefetch);
    gpsimd = weights + bulk x prefetch (SWDGE starts transfers ~100ns in)
    + out; scalar = wv + one startup x tile + final out half.
"""

import numpy as np

import concourse.bacc as bacc
import concourse.mybir as mybir
import concourse.tile as tile
from concourse import bass2jax

B, S, E, H = 8, 2048, 1024, 64
NCORES = 8
PB = 128  # partition block / k-tile / q-subtile size
ET = E // PB  # e-tiles per contraction
KT = S // PB  # k-tiles / q-subtiles

# q-chunk widths (must sum to S, each a multiple of PB, max 512)
WIDTHS = [512, 512, 512, 512]
CHUNKS = []
_c0 = 0
for _w in WIDTHS:
    CHUNKS.append((_c0, _w))
    _c0 += _w
assert _c0 == S
QC = len(CHUNKS)

# consts layout (f32 columns; fp16 payloads are bit-packed in pairs so the
# tensor stays NaN-free for finite checks)
C_QKB = 0  # [*, 0:1]   qk bias f32 (rows 0:64 bq/8, 64:128 bk)
C_MASK = 1  # [*, 1:65]  causal ramp strip, fp16 pairs: (p, j) = 1 iff j >= p
C_ONES = C_MASK + PB // 2  # [*, 65:66] fp16 ones pair
NCONST = C_ONES + 1

F32 = mybir.dt.float32
F16 = mybir.dt.float16
AF = mybir.ActivationFunctionType
MUL = mybir.AluOpType.mult

_CACHE: dict = {}

CFG = {
    "xbufs": 12,
    "wbufs": 10,
    "dma2": "gpsimd",
    # hold chunk-c x prefetch DMAs until ~these sim times (ms): keeps the
    # greedy scheduler from stuffing in-order DMA queues with prefetch
    # ahead of critical-path transfers
    "xwait": [0.0, 0.0, 0.002, 0.005, 0.009],
}


def _interleave(*gens):
    """Drive generators round-robin, one step per turn, in priority order."""
    alive = list(gens)
    while alive:
        for g in list(alive):
            try:
                next(g)
            except StopIteration:
                alive.remove(g)


def _build_nc():
    nc = bacc.Bacc("TRN2", target_bir_lowering=False, debug=False)
    xT = nc.dram_tensor("xT", [E, S], F16, kind="ExternalInput").ap()
    wqk = nc.dram_tensor("wqk", [E, PB], F16, kind="ExternalInput").ap()
    wv = nc.dram_tensor("wv", [E, H], F16, kind="ExternalInput").ap()
    consts = nc.dram_tensor("consts", [PB, NCONST], F32, kind="ExternalInput").ap()
    out = nc.dram_tensor("out", [S, H + 1], F32, kind="ExternalOutput").ap()

    with tile.TileContext(nc) as tc:
        with (
            tc.tile_pool(name="const", bufs=1) as constp,
            tc.tile_pool(name="xs", bufs=CFG["xbufs"]) as xpool,
            tc.tile_pool(name="qkv", bufs=1) as qkvp,
            tc.tile_pool(name="wt", bufs=CFG["wbufs"]) as wtp,
            tc.tile_pool(name="pqk", bufs=1, space="PSUM") as pqk,
            tc.tile_pool(name="pv", bufs=1, space="PSUM") as pvp,
            tc.tile_pool(name="ps", bufs=2, space="PSUM") as psp,
            tc.tile_pool(name="pavA", bufs=1, space="PSUM") as pavA,
            tc.tile_pool(name="pavB", bufs=1, space="PSUM") as pavB,
        ):
            wqk_sb = constp.tile([PB, ET, PB], F16)
            wv_sb = constp.tile([PB, ET, H], F16)
            cs = constp.tile([PB, NCONST], F32)
            # first weight slice gates the first matmul; gpsimd/SWDGE
            # starts transfers ~100ns into the kernel
            nc.gpsimd.dma_start(
                wqk_sb[:, 0:2, :],
                wqk[0 : 2 * PB, :].rearrange("(t p) m -> p t m", p=PB),
            )

            def load_weights_rest():
                nc.gpsimd.dma_start(
                    wqk_sb[:, 2:4, :],
                    wqk[2 * PB : 4 * PB, :].rearrange("(t p) m -> p t m", p=PB),
                )
                nc.gpsimd.dma_start(
                    wqk_sb[:, 4:ET, :],
                    wqk[4 * PB :, :].rearrange("(t p) m -> p t m", p=PB),
                )
                # held back so chunk-0's x tile wins the scalar queue head;
                # wv is not needed until the first v-flip chain (~4.5us)
                with tc.tile_wait_until(0.0023):
                    nc.scalar.dma_start(
                        wv_sb[:], wv[:].rearrange("(t p) m -> p t m", p=PB)
                    )

            qkbias_ap = cs[:, C_QKB : C_QKB + 1]
            mask_ap = cs[:, C_MASK : C_MASK + PB // 2].bitcast(F16)

            qk_sb = qkvp.tile([PB, S], F16)  # q at 0:64, k at 64:128
            kT = qkvp.tile([H, S], F16)  # k re-based to partitions 0:64
            vsb = qkvp.tile([PB, KT, H + 1], F16)  # v k-major + ones col
            osb = qkvp.tile([PB, KT, H + 1], F32)  # out staging

            def load_consts():
                # held back so chunk-0's x tile wins the sync queue head;
                # bias/mask aren't read until the first epilogue (~4.5us)
                with tc.tile_wait_until(0.0025):
                    nc.sync.dma_start(cs[:], consts[:])
                ones16 = cs[:, C_ONES : C_ONES + 1].bitcast(F16)
                nc.vector.tensor_copy(
                    vsb[:, :, H : H + 1],
                    ones16[:, 0:1, None].to_broadcast((PB, KT, 1)),
                )

            proj_state = {}

            def proj_main(ci):
                col0, W = CHUNKS[ci]
                qs = slice(col0, col0 + W)
                p_qk = pqk.tile([PB, W], F32, tag="pqk")
                xts = []
                with tc.tile_wait_until(CFG["xwait"][ci]):
                    for ep in range(ET // 2):
                        xt = xpool.tile([PB, 2, W], F16, tag="xt")
                        xts.append(xt)
                        if ci == 0:
                            dma_eng = (nc.sync, getattr(nc, CFG["dma2"]), nc.scalar, nc.sync)[ep]
                        elif ci == 1:
                            dma_eng = nc.sync if ep % 2 == 0 else getattr(nc, CFG["dma2"])
                        else:
                            dma_eng = getattr(nc, CFG["dma2"])
                        dma_eng.dma_start(
                            xt[:],
                            xT[ep * 2 * PB : (ep + 1) * 2 * PB, qs].rearrange(
                                "(t p) q -> p t q", p=PB
                            ),
                        )
                for ep in range(ET // 2):
                    for t in range(2):
                        e = 2 * ep + t
                        nc.tensor.matmul(
                            p_qk[:],
                            wqk_sb[:, e, :],
                            xts[ep][:, t, :],
                            start=(e == 0),
                            stop=(e == ET - 1),
                        )
                    yield
                proj_state[ci] = (p_qk, xts)

            def epi_qk(ci):
                col0, W = CHUNKS[ci]
                qs = slice(col0, col0 + W)
                p_qk, xts = proj_state[ci]
                if ci == 0:
                    # ACT is idle at startup (DVE does the ones-fill)
                    nc.scalar.activation(
                        qk_sb[:, qs], p_qk[:], AF.Identity, bias=qkbias_ap
                    )
                else:
                    nc.vector.tensor_scalar(
                        qk_sb[:, qs], p_qk[:], qkbias_ap, None,
                        mybir.AluOpType.add, mybir.AluOpType.bypass,
                    )
                # re-base k to partitions 0:64 for the scores matmul; only
                # this chunk's own (band) score tiles wait on it
                nc.sync.dma_start(kT[:, qs], qk_sb[H:PB, qs])

            def v_flip(ci):
                # flipped v projection: out [128 s, 64 h], x slice stationary
                col0, W = CHUNKS[ci]
                _, xts = proj_state[ci]
                for sub in range(W // PB):
                    m = col0 // PB + sub
                    p_v = pvp.tile([PB, H], F32, tag="pv")
                    for e in range(ET):
                        nc.tensor.matmul(
                            p_v[:],
                            xts[e // 2][:, e % 2, sub * PB : (sub + 1) * PB],
                            wv_sb[:, e, :],
                            start=(e == 0),
                            stop=(e == ET - 1),
                        )
                    nc.vector.tensor_copy(vsb[:, m, 0:H], p_v[:])
                    yield
                proj_state.pop(ci)

            def attn(ci):
                col0, W = CHUNKS[ci]
                nkt = (col0 + W) // PB  # k-tiles this chunk attends to
                nband = W // PB  # its own (diagonal band) k-tiles
                npair = nkt // 2
                kt0 = col0 // PB  # first band k-tile / q-subtile
                ws = {}

                def score_exp_pair(p, lo=0):
                    # pair p covers k-tiles (2p, 2p+1); lo>0 computes/exps
                    # only columns lo:W (band subrect)
                    p_s = psp.tile([PB, 2 * W], F32, tag="ps")
                    w = wtp.tile([PB, 2 * W], F16, tag="w")
                    for t in range(2):
                        m = 2 * p + t
                        nc.tensor.matmul(
                            p_s[:, t * W + lo : (t + 1) * W],
                            kT[:, m * PB : (m + 1) * PB],
                            qk_sb[0:H, col0 + lo : col0 + W],
                            start=True,
                            stop=True,
                        )
                    if lo:
                        nc.scalar.activation(
                            w[:].rearrange("p (t q) -> p t q", t=2)[:, :, lo:W],
                            p_s[:].rearrange("p (t q) -> p t q", t=2)[:, :, lo:W],
                            AF.Exp,
                        )
                    else:
                        nc.scalar.activation(w[:], p_s[:], AF.Exp)
                    ws[p] = w

                def strip(d):
                    # diagonal strip mask: first 128 valid cols of band tile d
                    m = kt0 + d
                    colo = (m % 2) * W + d * PB
                    w = ws[m // 2]
                    nc.vector.tensor_tensor(
                        w[:, colo : colo + PB],
                        w[:, colo : colo + PB],
                        mask_ap,
                        MUL,
                    )

                av_state = {}

                def av(sub, until=None):
                    # flipped AV: per q-subtile accumulation, out [128 q, 65].
                    # until=kt emits only tiles < kt (group left open); a
                    # later av(sub) call finishes and copies out.
                    s = kt0 + sub
                    if sub in av_state:
                        p_av, k_from = av_state.pop(sub)
                    else:
                        p_av = (pavA if sub % 2 == 0 else pavB).tile(
                            [PB, H + 1], F32, tag="pav"
                        )
                        k_from = 0
                    hi = s + 1 if until is None else until
                    for kt in range(k_from, hi):
                        w = ws[kt // 2]
                        colo = (kt % 2) * W + sub * PB
                        nc.tensor.matmul(
                            p_av[:],
                            w[:, colo : colo + PB],
                            vsb[:, kt, :],
                            start=(kt == 0),
                            stop=(kt == s),
                        )
                    if until is not None:
                        av_state[sub] = (p_av, hi)
                        return
                    nc.vector.tensor_copy(osb[:, s, :], p_av[:])

                # non-band pairs first (they only need OLD k-tiles), then
                # band pairs; for 512-wide chunks the last band pair is a
                # subrect. Strips/AV follow their pairs.
                for p in range(npair - nband // 2):
                    score_exp_pair(p)
                    yield
                if nband == 2:
                    score_exp_pair(npair - 1)
                    yield
                    strip(0)
                    strip(1)
                    yield
                    av(0)
                    yield
                    av(1)
                    yield
                else:
                    score_exp_pair(npair - 2)
                    strip(0)
                    strip(1)
                    yield
                    av(0)
                    yield
                    av(1)
                    yield
                    score_exp_pair(npair - 1, lo=W // 2)
                    strip(2)
                    strip(3)
                    yield
                    av(2)
                    yield
                    av(3)
                    yield
                # out DMA for this chunk (final chunk split for drain overlap)
                nsplit = 2 if ci == QC - 1 else 1
                nsub = W // PB
                for hh in range(nsplit):
                    r0 = col0 + hh * (W // nsplit)
                    r1 = col0 + (hh + 1) * (W // nsplit)
                    tl = slice(kt0 + hh * (nsub // nsplit),
                               kt0 + (hh + 1) * (nsub // nsplit))
                    if ci < QC - 1:
                        oq = nc.gpsimd
                    else:
                        oq = nc.sync if hh == 0 else nc.scalar
                    oq.dma_start(
                        out[r0:r1, :].rearrange("(t p) h -> p t h", p=PB),
                        osb[:, tl, :],
                    )
                    yield

            g0 = proj_main(0)
            next(g0)  # emits chunk-0 x DMAs + first matmul pair
            load_weights_rest()
            load_consts()
            _interleave(g0)
            epi_qk(0)
            for c in range(1, QC):
                _interleave(attn(c - 1), proj_main(c), v_flip(c - 1))
                epi_qk(c)
            _interleave(attn(QC - 1), v_flip(QC - 1))

    nc.compile()
    return nc


def _host_inputs(x, Wq, bq, Wk, bk, Wv, bv):
    x = np.asarray(x, np.float32)
    Wq, bq = np.asarray(Wq, np.float32), np.asarray(bq, np.float32)
    Wk, bk = np.asarray(Wk, np.float32), np.asarray(bk, np.float32)
    Wv = np.asarray(Wv, np.float32)

    sc = np.float32(1.0 / np.sqrt(H))
    wqk_h = np.concatenate([Wq.T * sc, Wk.T], axis=1).astype(np.float16)
    wvT = np.ascontiguousarray(Wv.T).astype(np.float16)

    cs = np.zeros((PB, NCONST), np.float32)
    cs[:, C_QKB] = np.concatenate([bq * sc, bk]).astype(np.float32)
    j = np.arange(PB)[None, :]
    p = np.arange(PB)[:, None]
    mask16 = (j >= p).astype(np.float16)  # [128, 128]
    cs[:, C_MASK : C_MASK + PB // 2] = mask16.view(np.float32)
    cs[:, C_ONES] = np.array([1.0, 1.0], np.float16).view(np.float32)[0]

    shared = {"wqk": np.ascontiguousarray(wqk_h), "wv": wvT, "consts": cs}
    in_maps = []
    for b in range(B):
        m = dict(shared)
        m["xT"] = np.ascontiguousarray(x[b].T.astype(np.float16))
        in_maps.append(m)
    return in_maps


def get_nc():
    if "nc" not in _CACHE:
        _CACHE["nc"] = _build_nc()
    return _CACHE["nc"]


def kernel(x, Wq, bq, Wk, bk, Wv, bv):
    nc = get_nc()
    in_maps = _host_inputs(x, Wq, bq, Wk, bk, Wv, bv)
    results = bass2jax.run_bass_via_pjrt(nc, in_maps, n_cores=NCORES)
    bv32 = np.asarray(bv, np.float32)
    out = np.empty((B, S, H), np.float32)
    for b in range(B):
        o = results[b]["out"]
        out[b] = o[:, :H] / o[:, H : H + 1] + bv32
    return out
